# revision 61
# baseline (speedup 1.0000x reference)
"""Multi-head attention TRN2 kernel, 8-core SPMD, globally software-pipelined.

Sharding: each core owns 1024 query rows (batch b = core//2, sequence half
core%2) and computes the full forward pass for those rows. No collectives.

One global pipeline over all 256 score chunks (16 heads x 16 key-chunks):
scores(c) / exp(c) / mask(c) stream ahead while PV(c - LAG) trails, so head
and pair boundaries never drain the scalar engine (exp is the per-core floor
at ~266us). Projection units for head-pair hp+1 are injected one per chunk
slot, filling the tensor engine between score matmuls.

All matmul operands are bf16 (f32 PSUM accumulation). Score chunks are
[128 k, 1024 q] in PSUM; exp runs on the scalar engine (scale 1/8, no max
subtraction -- scores/8 ~ N(0,1)), writing bf16; the 0/1 mask multiplies on
the vector engine (all-bf16 SBUF operands hit DVE fast modes). PV uses the
[65, q] accumulator orientation (lhsT = V chunk [128 k, 65], 65th column of
ones accumulates the softmax denominator); each [65, 512] accumulation group
owns a whole PSUM bank (groups are bank-granular). Normalization multiplies
by the broadcast reciprocal denominator, and an SBUF->SBUF DMA places each
head's X.T rows at the right partition offset of the resident x_all slab for
the output projection tail.

PSUM budget (8 banks): scores ring 2x2 banks, projection-partial ring 2x1,
PV accumulator [65, 1024] 1x2.
"""

from contextlib import ExitStack

import numpy as np

B, S, D, H, DH = 4, 2048, 1024, 16, 64
NQ = 1024          # query rows per core
NK = 2048          # keys per core (full sequence of its batch)
NCORES = 8
NP = 8             # head pairs

_CACHE = {}


def _build():
    from collections import deque

    import concourse.mybir as mybir
    import concourse.tile as tile
    from concourse import bacc

    f32 = mybir.dt.float32
    f32r = mybir.dt.float32r
    bf16 = mybir.dt.bfloat16
    EXP = mybir.ActivationFunctionType.Exp

    nc = bacc.Bacc(
        "TRN2",
        target_bir_lowering=False,
        debug=False,
        enable_asserts=False,
        num_devices=NCORES,
    )

    xq_d = nc.dram_tensor("xq_t", [D, NQ], bf16, kind="ExternalInput").ap()
    xk_d = nc.dram_tensor("xk_t", [D, NK], bf16, kind="ExternalInput").ap()
    xv_d = nc.dram_tensor("xv_t", [D, NK], bf16, kind="ExternalInput").ap()
    wq_d = nc.dram_tensor("wq_t", [NP, D, 128], bf16, kind="ExternalInput").ap()
    wk_d = nc.dram_tensor("wk_t", [NP, D, 128], bf16, kind="ExternalInput").ap()
    wv_d = nc.dram_tensor("wv_t", [NP, D, 128], bf16, kind="ExternalInput").ap()
    wo_d = nc.dram_tensor("wo_t", [8, 8, 128, 128], bf16, kind="ExternalInput").ap()
    mk_d = nc.dram_tensor("mask_t", [NK, NQ], bf16, kind="ExternalInput").ap()
    wbc_d = nc.dram_tensor("wb_cols", [128, 24], f32, kind="ExternalInput").ap()
    out_d = nc.dram_tensor("out_t", [D, NQ], bf16, kind="ExternalOutput").ap()

    def split8(ap_2d):
        # [1024, N] dram view -> [128, 8, N]: partition p, chunk i, col
        return ap_2d.rearrange("(i p) q -> p i q", p=128)

    with tile.TileContext(nc) as tc:
        stk = ExitStack()

        kpool = stk.enter_context(tc.tile_pool(name="konst", bufs=1))
        wbc = kpool.tile([128, 24], f32, name="wbc")
        nc.sync.dma_start(wbc[:], wbc_d[:, :])

        rpool = stk.enter_context(tc.tile_pool(name="resid", bufs=1))
        xk_sb = rpool.tile([128, 8, NK], bf16, name="xk_sb")
        xq_sb = rpool.tile([128, 8, NQ], bf16, name="xq_sb")
        xv_sb = rpool.tile([128, 8, NK], bf16, name="xv_sb")
        mask_sb = rpool.tile([128, 16, NQ], bf16, name="mask_sb")
        x_all = rpool.tile([128, 8, NQ], bf16, name="x_all")

        astk = ExitStack()
        wpool = astk.enter_context(tc.tile_pool(name="wring", bufs=2))
        kqpool = astk.enter_context(tc.tile_pool(name="kqring", bufs=3))
        vpool = astk.enter_context(tc.tile_pool(name="vring", bufs=3))
        pepool = astk.enter_context(tc.tile_pool(name="pering", bufs=12))
        xnpool = astk.enter_context(tc.tile_pool(name="xnring", bufs=2))
        rcpool = astk.enter_context(tc.tile_pool(name="rcring", bufs=1))
        stps = astk.enter_context(tc.tile_pool(name="stps", bufs=2, space="PSUM"))
        xtps = astk.enter_context(tc.tile_pool(name="xtps", bufs=1, space="PSUM"))

        slabs = {}

        def make_units(hp):
            """Projection work for pair hp as a list of closures; each emits
            <=0.5us of PE work (plus its DMA / consumer ops)."""
            state = {
                "k": kqpool.tile([128, NK], bf16, tag="k", name="k_slab"),
                "q": kqpool.tile([128, NQ], bf16, tag="q", name="q_slab"),
                "v": vpool.tile([128, 16, 2, 65], bf16, tag="v", name="v_slab"),
            }


            def dma_wv():
                w = wpool.tile([128, 8, 128], bf16, tag="wv", name="wv_sb")
                nc.sync.dma_start(w[:], split8(wv_d[hp, :, :]))
                state["wv"] = w
                nc.vector.memset(state["v"][:, :, :, 64:65], 1.0)

            def v_unit(kb):
                def run():
                    ps = stps.tile([128, 512], f32, tag="ps", name="ps_v")
                    for i in range(8):
                        nc.tensor.matmul(
                            ps[:, 0:128],
                            lhsT=xv_sb[:, i, kb * 128:(kb + 1) * 128],
                            rhs=state["wv"][:, i, :],
                            start=(i == 0),
                            stop=(i == 7),
                        )
                    for h2 in range(2):
                        nc.vector.tensor_copy(
                            state["v"][:, kb, h2, 0:64],
                            ps[:, h2 * 64:(h2 + 1) * 64],
                        )
                return run

            def dma_wk():
                w = wpool.tile([128, 8, 128], bf16, tag="wk", name="wk_sb")
                nc.sync.dma_start(w[:], split8(wk_d[hp, :, :]))
                state["wk"] = w

            def k_unit(kq, half):
                def run():
                    if half == 0:
                        state[f"psk{kq}"] = stps.tile(
                            [128, 512], f32, tag="ps", name="ps_k"
                        )
                    ps = state[f"psk{kq}"]
                    for i in range(4 * half, 4 * half + 4):
                        nc.tensor.matmul(
                            ps[:],
                            lhsT=state["wk"][:, i, :],
                            rhs=xk_sb[:, i, kq * 512:(kq + 1) * 512],
                            start=(i == 0),
                            stop=(i == 7),
                        )
                    if half == 1:
                        nc.vector.tensor_scalar_add(
                            state["k"][:, kq * 512:(kq + 1) * 512],
                            ps[:],
                            wbc[:, 8 + hp:9 + hp],
                        )
                return run

            def dma_wq():
                w = wpool.tile([128, 8, 128], bf16, tag="wq", name="wq_sb")
                nc.sync.dma_start(w[:], split8(wq_d[hp, :, :]))
                state["wq"] = w

            def q_unit(qh, half):
                def run():
                    if half == 0:
                        state[f"psq{qh}"] = stps.tile(
                            [128, 512], f32, tag="ps", name="ps_q"
                        )
                    ps = state[f"psq{qh}"]
                    for i in range(4 * half, 4 * half + 4):
                        nc.tensor.matmul(
                            ps[:],
                            lhsT=state["wq"][:, i, :],
                            rhs=xq_sb[:, i, qh * 512:(qh + 1) * 512],
                            start=(i == 0),
                            stop=(i == 7),
                        )
                    if half == 1:
                        nc.vector.tensor_scalar_add(
                            state["q"][:, qh * 512:(qh + 1) * 512],
                            ps[:],
                            wbc[:, hp:hp + 1],
                        )
                return run

            slabs[hp] = state
            kq_units = [k_unit(kq, half) for kq in range(4) for half in range(2)]
            kq_units += [q_unit(qh, half) for qh in range(2) for half in range(2)]
            v_units = [v_unit(kb) for kb in range(16)]
            return [dma_wk, dma_wq, dma_wv], kq_units, v_units

        LAG = 11
        CH = H * 16

        # DMA transfers serialize on one global device in issue order, so
        # issue strictly by first-need: pair-0 weight slabs, then xk (K
        # proj), xq, the first mask chunks (consumed progressively from the
        # first exp onward), then xv interleaved with the remaining mask.
        # Fill: pairs 0 and 1 project completely before attention starts,
        # so every engine-written stationary (V/K/Q slabs) is written well
        # before its first PE weight load -- engine completion semaphores
        # fire before SBUF writes become visible to the weight-load path, so
        # freshly written stationaries need a large real-time lead.
        inject = deque()
        dmas0, kq0, v0 = make_units(0)
        dmas1, kq1, v1 = make_units(1)
        for u in dmas0 + dmas1:
            u()
        for i in range(8):
            nc.sync.dma_start(xv_sb[:, i, :], xv_d[i * 128:(i + 1) * 128, :])
        for i in range(8):
            nc.sync.dma_start(xk_sb[:, i, :], xk_d[i * 128:(i + 1) * 128, :])
        for i in range(8):
            nc.sync.dma_start(xq_sb[:, i, :], xq_d[i * 128:(i + 1) * 128, :])
        for i in range(16):
            nc.sync.dma_start(mask_sb[:, i, :], mk_d[i * 128:(i + 1) * 128, :])
        for u in v0 + kq0 + kq1:
            u()
        inject.extend(v1)

        pe_hist = {}
        xts_of = {}

        for c in range(CH + LAG):
            if c < CH:
                head, kc = c // 16, c % 16
                hp, hh = head // 2, 1 - head % 2
                if kc == 0 and head % 2 == 0 and hp + 2 < NP:
                    dmas2, kq2, v2 = make_units(hp + 2)
                    inject.extend(dmas2)
                    inject.extend(kq2)
                    inject.extend(v2)
                ks = slabs[hp]["k"][hh * 64:(hh + 1) * 64, :]
                qs = slabs[hp]["q"][hh * 64:(hh + 1) * 64, :]
                st = stps.tile([128, NQ], f32, tag="st", name="st")
                nc.tensor.matmul(
                    st[:],
                    lhsT=ks[:, kc * 128:(kc + 1) * 128],
                    rhs=qs[:],
                    start=True,
                    stop=True,
                )
                if inject:
                    u = inject.popleft()
                    if u is not None:
                        u()
                pe = pepool.tile([128, NQ], bf16, tag="pe", name="pe")
                nc.scalar.activation(pe[:], st[:], EXP, scale=0.125)
                nc.vector.tensor_mul(pe[:], pe[:], mask_sb[:, kc, :])
                pe_hist[c] = pe
            if c >= LAG:
                cc = c - LAG
                head, kc = cc // 16, cc % 16
                hp, hh = head // 2, 1 - head % 2
                pe = pe_hist.pop(cc)
                if kc == 0:
                    # [65, 1024]: rows 0-63 = X.T for this head, row 64 = the
                    # softmax denominator. Each [65, 512] accumulation group
                    # owns a whole PSUM bank (groups are bank-granular).
                    xts_of[head] = xtps.tile(
                        [65, NQ], f32, tag="xt", name="xt_ps", bufs=1
                    )
                xt = xts_of[head]
                for qh in range(2):
                    nc.tensor.matmul(
                        xt[:, qh * 512:(qh + 1) * 512],
                        lhsT=slabs[hp]["v"][:, kc, hh, :],
                        rhs=pe[:, qh * 512:(qh + 1) * 512],
                        start=(kc == 0),
                        stop=(kc == 15),
                    )
                if kc == 15:
                    # Free the PSUM accumulator fast (recip + one DVE copy),
                    # so the next head's PV is not gated on the slow
                    # broadcast; normalization continues off critical path.
                    xt = xts_of.pop(head)
                    rc = rcpool.tile([1, NQ], f32, tag="rc", name="rc")
                    nc.vector.reciprocal(rc[:], xt[64:65, :])
                    xraw = xnpool.tile(
                        [64, NQ], bf16, tag="xraw", name="xraw", bufs=1
                    )
                    nc.vector.tensor_copy(xraw[:], xt[0:64, :])
                    bc = rcpool.tile([64, NQ], f32, tag="bc", name="bc")
                    nc.gpsimd.partition_broadcast(bc[:], rc[:])
                    xb = xnpool.tile([64, NQ], bf16, tag="xb", name="xb", bufs=1)
                    nc.vector.tensor_mul(xb[:], xraw[:], bc[:])
                    nc.sync.dma_start(
                        x_all[hh * 64:(hh + 1) * 64, hp, :], xb[:]
                    )
        astk.close()

        # Tail: output projection out.T[f, q] = sum_d Wo.T[d, f] * X[d, q].
        with (
            tc.tile_pool(name="wo_ring", bufs=3) as wopool,
            tc.tile_pool(name="oc_ring", bufs=3) as ocpool,
            tc.tile_pool(name="ops", bufs=2, space="PSUM") as opool,
        ):
            for fb in range(8):
                wo_sb = wopool.tile([128, 8, 128], bf16, tag="wo", name="wo_sb")
                nc.sync.dma_start(
                    wo_sb[:], wo_d[fb].rearrange("i p f -> p i f")
                )
                for qh in range(2):
                    op = opool.tile([128, 512], f32, tag="op", name="op")
                    for dp in range(8):
                        nc.tensor.matmul(
                            op[:],
                            lhsT=wo_sb[:, dp, :],
                            rhs=x_all[:, dp, qh * 512:(qh + 1) * 512],
                            start=(dp == 0),
                            stop=(dp == 7),
                        )
                    outc = ocpool.tile([128, 512], bf16, tag="oc", name="outc")
                    nc.vector.tensor_scalar_add(
                        outc[:], op[:], wbc[:, 16 + fb:17 + fb]
                    )
                    nc.sync.dma_start(
                        out_d[fb * 128:(fb + 1) * 128, qh * 512:(qh + 1) * 512],
                        outc[:],
                    )
        stk.close()

    nc.compile()
    return nc


def _get_nc():
    if "nc" not in _CACHE:
        _CACHE["nc"] = _build()
    return _CACHE["nc"]


def _prep(query, key, value, mask, Wq, bq, Wk, bk, Wv, bv, Wo, bo):
    import ml_dtypes

    f = np.float32
    bf = ml_dtypes.bfloat16

    def wt_tiles(W):  # W [D, D] -> [8, D, 128] slices of W.T along fout
        WT = np.ascontiguousarray(np.asarray(W, dtype=f).T)
        return np.ascontiguousarray(
            np.stack([WT[:, i * 128:(i + 1) * 128] for i in range(8)], 0)
        ).astype(bf)

    wq_t = wt_tiles(Wq)
    wk_t = wt_tiles(Wk)
    wv_t = wt_tiles(Wv)
    WoT = np.ascontiguousarray(np.asarray(Wo, dtype=f).T)
    wo_t = np.ascontiguousarray(
        np.stack(
            [
                np.stack(
                    [
                        WoT[dp * 128:(dp + 1) * 128, fb * 128:(fb + 1) * 128]
                        for dp in range(8)
                    ],
                    0,
                )
                for fb in range(8)
            ],
            0,
        )
    ).astype(bf)
    bo_eff = (
        np.asarray(bo, dtype=np.float64)
        + np.asarray(Wo, dtype=np.float64) @ np.asarray(bv, dtype=np.float64)
    ).astype(f)
    wb_cols = np.stack(
        [np.asarray(b).astype(f).reshape(8, 128).T for b in (bq, bk, bo_eff)],
        1,
    ).reshape(128, 24)
    wb_cols = np.ascontiguousarray(wb_cols)
    m2 = np.asarray(mask)[0, 0]  # [S, S] int
    in_maps = []
    for c in range(NCORES):
        b, half = c // 2, c % 2
        qsl = slice(half * NQ, (half + 1) * NQ)
        in_maps.append(
            {
                "xq_t": np.ascontiguousarray(
                    np.asarray(query)[b, qsl].T.astype(bf)
                ),
                "xk_t": np.ascontiguousarray(np.asarray(key)[b].T.astype(bf)),
                "xv_t": np.ascontiguousarray(np.asarray(value)[b].T.astype(bf)),
                "wq_t": wq_t,
                "wk_t": wk_t,
                "wv_t": wv_t,
                "wo_t": wo_t,
                "wb_cols": wb_cols,
                "mask_t": np.ascontiguousarray(m2[qsl, :].T).astype(bf),
            }
        )
    return in_maps


def kernel(**inputs):
    from concourse.bass_utils import run_bass_kernel_spmd

    np_inputs = {k: np.asarray(v) for k, v in inputs.items()}
    in_maps = _prep(**np_inputs)
    nc = _get_nc()
    res = run_bass_kernel_spmd(nc, in_maps, list(range(NCORES)))
    out = np.empty((B, S, D), np.float32)
    for c in range(NCORES):
        b, half = c // 2, c % 2
        out[b, half * NQ:(half + 1) * NQ, :] = (
            res.results[c]["out_t"].astype(np.float32).T
        )
    return out


# revision 62
# speedup vs baseline: 1.0004x; 1.0004x over previous
"""Multi-head attention TRN2 kernel, 8-core SPMD, globally software-pipelined.

Sharding: each core owns 1024 query rows (batch b = core//2, sequence half
core%2) and computes the full forward pass for those rows. No collectives.

One global pipeline over all 256 score chunks (16 heads x 16 key-chunks):
scores(c) / exp(c) / mask(c) stream ahead while PV(c - LAG) trails, so head
and pair boundaries never drain the scalar engine (exp is the per-core floor
at ~266us). Projection units for head-pair hp+1 are injected one per chunk
slot, filling the tensor engine between score matmuls.

All matmul operands are bf16 (f32 PSUM accumulation). Score chunks are
[128 k, 1024 q] in PSUM; exp runs on the scalar engine (scale 1/8, no max
subtraction -- scores/8 ~ N(0,1)), writing bf16; the 0/1 mask multiplies on
the vector engine (all-bf16 SBUF operands hit DVE fast modes). PV uses the
[65, q] accumulator orientation (lhsT = V chunk [128 k, 65], 65th column of
ones accumulates the softmax denominator); each [65, 512] accumulation group
owns a whole PSUM bank (groups are bank-granular). Normalization multiplies
by the broadcast reciprocal denominator, and an SBUF->SBUF DMA places each
head's X.T rows at the right partition offset of the resident x_all slab for
the output projection tail.

PSUM budget (8 banks): scores ring 2x2 banks, projection-partial ring 2x1,
PV accumulator [65, 1024] 1x2.
"""

from contextlib import ExitStack

import numpy as np

B, S, D, H, DH = 4, 2048, 1024, 16, 64
NQ = 1024          # query rows per core
NK = 2048          # keys per core (full sequence of its batch)
NCORES = 8
NP = 8             # head pairs

_CACHE = {}


def _build():
    from collections import deque

    import concourse.mybir as mybir
    import concourse.tile as tile
    from concourse import bacc

    f32 = mybir.dt.float32
    f32r = mybir.dt.float32r
    bf16 = mybir.dt.bfloat16
    EXP = mybir.ActivationFunctionType.Exp

    nc = bacc.Bacc(
        "TRN2",
        target_bir_lowering=False,
        debug=False,
        enable_asserts=False,
        num_devices=NCORES,
    )

    xq_d = nc.dram_tensor("xq_t", [D, NQ], bf16, kind="ExternalInput").ap()
    xk_d = nc.dram_tensor("xk_t", [D, NK], bf16, kind="ExternalInput").ap()
    xv_d = nc.dram_tensor("xv_t", [D, NK], bf16, kind="ExternalInput").ap()
    wq_d = nc.dram_tensor("wq_t", [NP, D, 128], bf16, kind="ExternalInput").ap()
    wk_d = nc.dram_tensor("wk_t", [NP, D, 128], bf16, kind="ExternalInput").ap()
    wv_d = nc.dram_tensor("wv_t", [NP, D, 128], bf16, kind="ExternalInput").ap()
    wo_d = nc.dram_tensor("wo_t", [8, 8, 128, 128], bf16, kind="ExternalInput").ap()
    mk_d = nc.dram_tensor("mask_t", [NK, NQ], bf16, kind="ExternalInput").ap()
    wbc_d = nc.dram_tensor("wb_cols", [128, 24], f32, kind="ExternalInput").ap()
    out_d = nc.dram_tensor("out_t", [D, NQ], bf16, kind="ExternalOutput").ap()

    def split8(ap_2d):
        # [1024, N] dram view -> [128, 8, N]: partition p, chunk i, col
        return ap_2d.rearrange("(i p) q -> p i q", p=128)

    with tile.TileContext(nc) as tc:
        stk = ExitStack()

        kpool = stk.enter_context(tc.tile_pool(name="konst", bufs=1))
        wbc = kpool.tile([128, 24], f32, name="wbc")
        nc.sync.dma_start(wbc[:], wbc_d[:, :])

        rpool = stk.enter_context(tc.tile_pool(name="resid", bufs=1))
        xk_sb = rpool.tile([128, 8, NK], bf16, name="xk_sb")
        xq_sb = rpool.tile([128, 8, NQ], bf16, name="xq_sb")
        xv_sb = rpool.tile([128, 8, NK], bf16, name="xv_sb")
        mask_sb = rpool.tile([128, 16, NQ], bf16, name="mask_sb")
        x_all = rpool.tile([128, 8, NQ], bf16, name="x_all")

        astk = ExitStack()
        wpool = astk.enter_context(tc.tile_pool(name="wring", bufs=2))
        kqpool = astk.enter_context(tc.tile_pool(name="kqring", bufs=3))
        vpool = astk.enter_context(tc.tile_pool(name="vring", bufs=3))
        pepool = astk.enter_context(tc.tile_pool(name="pering", bufs=12))
        xnpool = astk.enter_context(tc.tile_pool(name="xnring", bufs=2))
        rcpool = astk.enter_context(tc.tile_pool(name="rcring", bufs=1))
        stps = astk.enter_context(tc.tile_pool(name="stps", bufs=2, space="PSUM"))
        xtps = astk.enter_context(tc.tile_pool(name="xtps", bufs=1, space="PSUM"))

        slabs = {}

        def make_units(hp):
            """Projection work for pair hp as a list of closures; each emits
            <=0.5us of PE work (plus its DMA / consumer ops)."""
            state = {
                "k": kqpool.tile([128, NK], bf16, tag="k", name="k_slab"),
                "q": kqpool.tile([128, NQ], bf16, tag="q", name="q_slab"),
                "v": vpool.tile([128, 16, 2, 65], bf16, tag="v", name="v_slab"),
            }


            def dma_wv():
                w = wpool.tile([128, 8, 128], bf16, tag="wv", name="wv_sb")
                nc.sync.dma_start(w[:], split8(wv_d[hp, :, :]))
                state["wv"] = w
                nc.vector.memset(state["v"][:, :, :, 64:65], 1.0)

            def v_unit(kb):
                def run():
                    ps = stps.tile([128, 512], f32, tag="ps", name="ps_v")
                    for i in range(8):
                        nc.tensor.matmul(
                            ps[:, 0:128],
                            lhsT=xv_sb[:, i, kb * 128:(kb + 1) * 128],
                            rhs=state["wv"][:, i, :],
                            start=(i == 0),
                            stop=(i == 7),
                        )
                    for h2 in range(2):
                        nc.vector.tensor_copy(
                            state["v"][:, kb, h2, 0:64],
                            ps[:, h2 * 64:(h2 + 1) * 64],
                        )
                return run

            def dma_wk():
                w = wpool.tile([128, 8, 128], bf16, tag="wk", name="wk_sb")
                nc.sync.dma_start(w[:], split8(wk_d[hp, :, :]))
                state["wk"] = w

            def k_unit(kq, half):
                def run():
                    if half == 0:
                        state[f"psk{kq}"] = stps.tile(
                            [128, 512], f32, tag="ps", name="ps_k"
                        )
                    ps = state[f"psk{kq}"]
                    for i in range(4 * half, 4 * half + 4):
                        nc.tensor.matmul(
                            ps[:],
                            lhsT=state["wk"][:, i, :],
                            rhs=xk_sb[:, i, kq * 512:(kq + 1) * 512],
                            start=(i == 0),
                            stop=(i == 7),
                        )
                    if half == 1:
                        nc.vector.tensor_scalar_add(
                            state["k"][:, kq * 512:(kq + 1) * 512],
                            ps[:],
                            wbc[:, 8 + hp:9 + hp],
                        )
                return run

            def dma_wq():
                w = wpool.tile([128, 8, 128], bf16, tag="wq", name="wq_sb")
                nc.sync.dma_start(w[:], split8(wq_d[hp, :, :]))
                state["wq"] = w

            def q_unit(qh, half):
                def run():
                    if half == 0:
                        state[f"psq{qh}"] = stps.tile(
                            [128, 512], f32, tag="ps", name="ps_q"
                        )
                    ps = state[f"psq{qh}"]
                    for i in range(4 * half, 4 * half + 4):
                        nc.tensor.matmul(
                            ps[:],
                            lhsT=state["wq"][:, i, :],
                            rhs=xq_sb[:, i, qh * 512:(qh + 1) * 512],
                            start=(i == 0),
                            stop=(i == 7),
                        )
                    if half == 1:
                        nc.vector.tensor_scalar_add(
                            state["q"][:, qh * 512:(qh + 1) * 512],
                            ps[:],
                            wbc[:, hp:hp + 1],
                        )
                return run

            slabs[hp] = state
            kq_units = [k_unit(kq, half) for kq in range(4) for half in range(2)]
            kq_units += [q_unit(qh, half) for qh in range(2) for half in range(2)]
            v_units = [v_unit(kb) for kb in range(16)]
            return [dma_wk, dma_wq, dma_wv], kq_units, v_units

        LAG = 11
        CH = H * 16

        # DMA transfers serialize on one global device in issue order, so
        # issue strictly by first-need: pair-0 weight slabs, then xk (K
        # proj), xq, the first mask chunks (consumed progressively from the
        # first exp onward), then xv interleaved with the remaining mask.
        # Fill: pairs 0 and 1 project completely before attention starts,
        # so every engine-written stationary (V/K/Q slabs) is written well
        # before its first PE weight load -- engine completion semaphores
        # fire before SBUF writes become visible to the weight-load path, so
        # freshly written stationaries need a large real-time lead.
        inject = deque()
        dmas0, kq0, v0 = make_units(0)
        dmas1, kq1, v1 = make_units(1)
        for u in dmas0 + dmas1:
            u()
        for i in range(8):
            nc.sync.dma_start(xv_sb[:, i, :], xv_d[i * 128:(i + 1) * 128, :])
        for i in range(8):
            nc.sync.dma_start(xk_sb[:, i, :], xk_d[i * 128:(i + 1) * 128, :])
        for i in range(8):
            nc.sync.dma_start(xq_sb[:, i, :], xq_d[i * 128:(i + 1) * 128, :])
        for i in range(16):
            nc.sync.dma_start(mask_sb[:, i, :], mk_d[i * 128:(i + 1) * 128, :])
        for u in v0 + kq0 + kq1:
            u()
        inject.extend(v1)

        pe_hist = {}
        xts_of = {}

        for c in range(CH + LAG):
            if c < CH:
                head, kc = c // 16, c % 16
                hp, hh = head // 2, 1 - head % 2
                if kc == 0 and head % 2 == 0 and hp + 2 < NP:
                    dmas2, kq2, v2 = make_units(hp + 2)
                    inject.extend(dmas2)
                    inject.extend(kq2)
                    inject.extend(v2)
                ks = slabs[hp]["k"][hh * 64:(hh + 1) * 64, :]
                qs = slabs[hp]["q"][hh * 64:(hh + 1) * 64, :]
                st = stps.tile([128, NQ], f32, tag="st", name="st")
                for qh in range(2):
                    nc.tensor.matmul(
                        st[:, qh * 512:(qh + 1) * 512],
                        lhsT=ks[:, kc * 128:(kc + 1) * 128],
                        rhs=qs[:, qh * 512:(qh + 1) * 512],
                        start=True,
                        stop=True,
                    )
                if inject:
                    u = inject.popleft()
                    if u is not None:
                        u()
                pe = pepool.tile([128, NQ], bf16, tag="pe", name="pe")
                nc.scalar.activation(pe[:], st[:], EXP, scale=0.125)
                nc.vector.tensor_mul(pe[:], pe[:], mask_sb[:, kc, :])
                pe_hist[c] = pe
            if c >= LAG:
                cc = c - LAG
                head, kc = cc // 16, cc % 16
                hp, hh = head // 2, 1 - head % 2
                pe = pe_hist.pop(cc)
                if kc == 0:
                    # [65, 1024]: rows 0-63 = X.T for this head, row 64 = the
                    # softmax denominator. Each [65, 512] accumulation group
                    # owns a whole PSUM bank (groups are bank-granular).
                    xts_of[head] = xtps.tile(
                        [65, NQ], f32, tag="xt", name="xt_ps", bufs=1
                    )
                xt = xts_of[head]
                for qh in range(2):
                    nc.tensor.matmul(
                        xt[:, qh * 512:(qh + 1) * 512],
                        lhsT=slabs[hp]["v"][:, kc, hh, :],
                        rhs=pe[:, qh * 512:(qh + 1) * 512],
                        start=(kc == 0),
                        stop=(kc == 15),
                    )
                if kc == 15:
                    # Free the PSUM accumulator fast (recip + one DVE copy),
                    # so the next head's PV is not gated on the slow
                    # broadcast; normalization continues off critical path.
                    xt = xts_of.pop(head)
                    rc = rcpool.tile([1, NQ], f32, tag="rc", name="rc")
                    nc.vector.reciprocal(rc[:], xt[64:65, :])
                    xraw = xnpool.tile(
                        [64, NQ], bf16, tag="xraw", name="xraw", bufs=1
                    )
                    nc.vector.tensor_copy(xraw[:], xt[0:64, :])
                    bc = rcpool.tile([64, NQ], f32, tag="bc", name="bc")
                    nc.gpsimd.partition_broadcast(bc[:], rc[:])
                    xb = xnpool.tile([64, NQ], bf16, tag="xb", name="xb", bufs=1)
                    nc.vector.tensor_mul(xb[:], xraw[:], bc[:])
                    nc.sync.dma_start(
                        x_all[hh * 64:(hh + 1) * 64, hp, :], xb[:]
                    )
        astk.close()

        # Tail: output projection out.T[f, q] = sum_d Wo.T[d, f] * X[d, q].
        with (
            tc.tile_pool(name="wo_ring", bufs=3) as wopool,
            tc.tile_pool(name="oc_ring", bufs=3) as ocpool,
            tc.tile_pool(name="ops", bufs=2, space="PSUM") as opool,
        ):
            for fb in range(8):
                wo_sb = wopool.tile([128, 8, 128], bf16, tag="wo", name="wo_sb")
                nc.sync.dma_start(
                    wo_sb[:], wo_d[fb].rearrange("i p f -> p i f")
                )
                for qh in range(2):
                    op = opool.tile([128, 512], f32, tag="op", name="op")
                    for dp in range(8):
                        nc.tensor.matmul(
                            op[:],
                            lhsT=wo_sb[:, dp, :],
                            rhs=x_all[:, dp, qh * 512:(qh + 1) * 512],
                            start=(dp == 0),
                            stop=(dp == 7),
                        )
                    outc = ocpool.tile([128, 512], bf16, tag="oc", name="outc")
                    nc.vector.tensor_scalar_add(
                        outc[:], op[:], wbc[:, 16 + fb:17 + fb]
                    )
                    nc.sync.dma_start(
                        out_d[fb * 128:(fb + 1) * 128, qh * 512:(qh + 1) * 512],
                        outc[:],
                    )
        stk.close()

    nc.compile()
    return nc


def _get_nc():
    if "nc" not in _CACHE:
        _CACHE["nc"] = _build()
    return _CACHE["nc"]


def _prep(query, key, value, mask, Wq, bq, Wk, bk, Wv, bv, Wo, bo):
    import ml_dtypes

    f = np.float32
    bf = ml_dtypes.bfloat16

    def wt_tiles(W):  # W [D, D] -> [8, D, 128] slices of W.T along fout
        WT = np.ascontiguousarray(np.asarray(W, dtype=f).T)
        return np.ascontiguousarray(
            np.stack([WT[:, i * 128:(i + 1) * 128] for i in range(8)], 0)
        ).astype(bf)

    wq_t = wt_tiles(Wq)
    wk_t = wt_tiles(Wk)
    wv_t = wt_tiles(Wv)
    WoT = np.ascontiguousarray(np.asarray(Wo, dtype=f).T)
    wo_t = np.ascontiguousarray(
        np.stack(
            [
                np.stack(
                    [
                        WoT[dp * 128:(dp + 1) * 128, fb * 128:(fb + 1) * 128]
                        for dp in range(8)
                    ],
                    0,
                )
                for fb in range(8)
            ],
            0,
        )
    ).astype(bf)
    bo_eff = (
        np.asarray(bo, dtype=np.float64)
        + np.asarray(Wo, dtype=np.float64) @ np.asarray(bv, dtype=np.float64)
    ).astype(f)
    wb_cols = np.stack(
        [np.asarray(b).astype(f).reshape(8, 128).T for b in (bq, bk, bo_eff)],
        1,
    ).reshape(128, 24)
    wb_cols = np.ascontiguousarray(wb_cols)
    m2 = np.asarray(mask)[0, 0]  # [S, S] int
    in_maps = []
    for c in range(NCORES):
        b, half = c // 2, c % 2
        qsl = slice(half * NQ, (half + 1) * NQ)
        in_maps.append(
            {
                "xq_t": np.ascontiguousarray(
                    np.asarray(query)[b, qsl].T.astype(bf)
                ),
                "xk_t": np.ascontiguousarray(np.asarray(key)[b].T.astype(bf)),
                "xv_t": np.ascontiguousarray(np.asarray(value)[b].T.astype(bf)),
                "wq_t": wq_t,
                "wk_t": wk_t,
                "wv_t": wv_t,
                "wo_t": wo_t,
                "wb_cols": wb_cols,
                "mask_t": np.ascontiguousarray(m2[qsl, :].T).astype(bf),
            }
        )
    return in_maps


def kernel(**inputs):
    from concourse.bass_utils import run_bass_kernel_spmd

    np_inputs = {k: np.asarray(v) for k, v in inputs.items()}
    in_maps = _prep(**np_inputs)
    nc = _get_nc()
    res = run_bass_kernel_spmd(nc, in_maps, list(range(NCORES)))
    out = np.empty((B, S, D), np.float32)
    for c in range(NCORES):
        b, half = c // 2, c % 2
        out[b, half * NQ:(half + 1) * NQ, :] = (
            res.results[c]["out_t"].astype(np.float32).T
        )
    return out
